# revision 32
# baseline (speedup 1.0000x reference)
"""Trainium2 Bass kernel for nn_GatedCrossAttention.

Computes, for q,k of shape (B=64, D=1024) and weights Wq,Wk (D,D), Wg (D,2D):
    q_proj = q @ Wq.T + bq
    k_proj = k @ Wk.T + bk
    scores[b,i,j]   = q_proj[b,i] * k_proj[b,j]
    gate_pre[b,i,j] = q_proj[b,i] * w1s[j] + t[b,j]
       with w1s = Wg[:, :D].sum(1),  t = k_proj @ W2.T + bg,  W2 = Wg[:, D:]
    out = softmax_j(scores * sigmoid(sigmoid(gate_pre)))

Sharding: pure data parallel, 8 batches per core on 8 NeuronCores.

Per-core device algorithm (per 128-row chunk of one batch's 1024x1024 matrix):
    PE  : gate_pre = K=6 bf16 hi/lo outer-product matmul -> PSUM
    ACT : u = tanh(0.5*gate_pre)          [sigmoid(x) = 0.5 + 0.5*tanh(x/2)]
    DVE : y = M(u) * k_bcast              [custom fused op; M monic cubic]
    ACT : e = exp(qa_i * y), accum z = sum_j(e)
          [qa_i = a*q_proj_i; a*M(u) ~= sigmoid(sigmoid(.)); q_i folded into
           the per-partition activation scale so scores stay exact f32]
    DVE : out = e * (1/z)
Both ACT functions (tanh, exp) live in one table set ("exp_and_others"): no
table switches.  Softmax max-subtraction is skipped: exp args bounded (<~8).
"""

import sys

for _p in ("/opt/trn_rl_repo",):
    if _p not in sys.path:
        sys.path.append(_p)

import numpy as np

B = 64
D = 1024
NCORES = 8
BLOC = B // NCORES  # 8 batches per core

# --- cubic fit:  sigmoid(0.5 + 0.5*u) ~= A3 * (((u + CC0)*u + CC1)*u + CC2) on [-1,1]
# max abs error ~3.6e-5 (Lawson-iterated minimax, fitted offline).
_P0, _P1, _P2, _P3 = (
    0.6224234076915138,
    0.11748147912979392,
    -0.006919796246243861,
    -0.0019515843371938285,
)
A3 = _P3
CC0 = _P2 / _P3
CC1 = _P1 / _P3
CC2 = _P0 / _P3

_CACHE = {}
TRACE = False
LAST_RESULTS = None


def _make_sigmul_op():
    """Custom DVE op:  out = (((Src0 + C0)*Src0 + C1)*Src0 + C2) * Src1.

    Registered in concourse.dve_ops.OPS (the designed extension point) so the
    per-NEFF DVE table generation picks it up."""
    import concourse.dve_ops as dve_ops
    from concourse.dve_ops import DveOp
    from concourse.dve_spec import C0, C1, C2, Spec, Src0, Src1, lower
    from concourse.dve_uop import DveOpSpec

    NAME = "SIGMUL3_GCA"
    for op in dve_ops.OPS:
        if op.name == NAME:
            return op

    def _ref(in0, in1, s0, s1, imm2):
        x = in0.astype(np.float32)
        return ((((x + s0) * x + s1) * x + imm2) * in1).astype(np.float32)

    spec = Spec(
        body=(((Src0 + C0) * Src0 + C1) * Src0 + C2) * Src1,
        reference=_ref,
    )
    opcode = dve_ops._CUSTOM_DVE_ROW_BASE + len(dve_ops.OPS)
    assert opcode < 0x20
    shas = {}
    for ver in ("v3", "v4"):
        tmp = DveOpSpec(
            name=NAME, opcode=opcode, uops=lower(spec, ver=ver), rd1_en=True
        )
        shas[ver] = tmp.sha(ver)
    op = DveOp(NAME, spec, subdim=False, uops_sha=shas)
    dve_ops.OPS.append(op)
    dve_ops._SUB_OPCODE_FOR_NAME[NAME] = opcode
    dve_ops.CUSTOM_DVE_SPECS[NAME] = spec
    return op


def _build():
    import concourse.bacc as bacc
    import concourse.mybir as mybir
    import concourse.tile as tile

    f32 = mybir.dt.float32
    bf16 = mybir.dt.bfloat16
    AF = mybir.ActivationFunctionType
    sigmul = _make_sigmul_op()

    nc = bacc.Bacc(
        "TRN2",
        target_bir_lowering=False,
        debug=False,
        num_devices=NCORES,
    )

    # ---- DRAM I/O ----
    # qT/kT host-prearranged to the SBUF tile layout [p, kc, b]
    qT = nc.dram_tensor("qT", [128, (D // 128) * BLOC], f32, kind="ExternalInput")
    kT = nc.dram_tensor("kT", [128, (D // 128) * BLOC], f32, kind="ExternalInput")
    kTb = nc.dram_tensor("kTb", [128, (D // 128) * BLOC], bf16, kind="ExternalInput")
    WqT = nc.dram_tensor("WqT", [D, D], f32, kind="ExternalInput")
    WkT = nc.dram_tensor("WkT", [D, D], f32, kind="ExternalInput")
    # (W2 @ Wk).T in bf16: feeds only the error-tolerant gate path
    WtT = nc.dram_tensor("WtT", [D, D], bf16, kind="ExternalInput")
    w1sh = nc.dram_tensor("w1sh", [1, D], bf16, kind="ExternalInput")
    w1sl = nc.dram_tensor("w1sl", [1, D], bf16, kind="ExternalInput")
    ones8k = nc.dram_tensor("ones8k", [1, BLOC * D], bf16, kind="ExternalInput")
    bq = nc.dram_tensor("bq", [1, D], f32, kind="ExternalInput")
    bk = nc.dram_tensor("bk", [1, D], f32, kind="ExternalInput")
    bt = nc.dram_tensor("bt", [1, D], f32, kind="ExternalInput")  # bk@W2.T + bg
    out_d = nc.dram_tensor("out", [BLOC, D, D], f32, kind="ExternalOutput")

    NK = D // 128  # 8 row chunks / contraction chunks

    with tile.TileContext(nc) as tc:
        with (
            tc.tile_pool(name="spool", bufs=1) as spool,
            tc.tile_pool(name="dpool", bufs=1, space="DRAM") as dpool,
        ):
            # [qp | tp] share one tile so the hi/lo split runs as wide ops
            pt_sb = spool.tile([BLOC, 2 * D], f32, tag="pt")
            kp_sb = spool.tile([BLOC, D], f32, tag="kp")
            with (
                tc.tile_pool(name="wpool", bufs=1) as wpool,
                tc.tile_pool(name="wstream", bufs=3) as wstream,
                tc.tile_pool(name="ppool", bufs=3, space="PSUM") as ppool,
            ):
                # ---- load small inputs (idle queues) ----
                qT_sb = wpool.tile([128, NK, BLOC], f32, tag="qT")
                nc.gpsimd.dma_start(
                    qT_sb[:], qT[:].rearrange("p (n b) -> p n b", n=NK)
                )
                kT_sb = wpool.tile([128, NK, BLOC], f32, tag="kT")
                nc.gpsimd.dma_start(
                    kT_sb[:], kT[:].rearrange("p (n b) -> p n b", n=NK)
                )
                kTb_sb = wpool.tile([128, NK, BLOC], bf16, tag="kTb")
                nc.gpsimd.dma_start(
                    kTb_sb[:], kTb[:].rearrange("p (n b) -> p n b", n=NK)
                )

                b_sbs = []
                for nm, dram in (("bq", bq), ("bk", bk), ("bt", bt)):
                    b_sb = wpool.tile([1, D], f32, tag=nm)
                    nc.gpsimd.dma_start(b_sb[:], dram[:])
                    b_sbs.append(b_sb)
                bq_sb, bk_sb, bt_sb = b_sbs

                ones1 = wpool.tile([1, BLOC], f32, tag="ones1")
                nc.vector.memset(ones1[:], 1.0)

                # ---- projections: proj = x @ W.T + b ----
                # weights streamed in 128-row chunks, triple-buffered
                for nm, xT_sb, w_dram, wdt, b_sb, dma_eng, dst in (
                    ("qp", qT_sb, WqT, f32, bq_sb, nc.sync, pt_sb[:, 0:D]),
                    ("kp", kT_sb, WkT, f32, bk_sb, nc.scalar, kp_sb[:]),
                    ("tp", kTb_sb, WtT, bf16, bt_sb, nc.gpsimd, pt_sb[:, D : 2 * D]),
                ):
                    ps = ppool.tile([BLOC, D], f32, tag="proj_ps")
                    for kc in range(NK):
                        wch = wstream.tile([128, D], wdt, tag="wc" + nm)
                        dma_eng.dma_start(
                            wch[:], w_dram[128 * kc : 128 * kc + 128, :]
                        )
                        for nb in range(2):
                            sl = slice(512 * nb, 512 * nb + 512)
                            nc.tensor.matmul(
                                ps[:, sl],
                                xT_sb[:, kc, :],
                                wch[:, sl],
                                start=(kc == 0),
                                stop=False,
                            )
                    for nb in range(2):
                        sl = slice(512 * nb, 512 * nb + 512)
                        nc.tensor.matmul(
                            ps[:, sl], ones1[:], b_sb[:, sl], start=False, stop=True
                        )
                    nc.vector.tensor_copy(dst, ps[:])

            # ---- bf16 hi/lo split of [qp | tp] with two wide ops (PE
            # multiplies bf16 inputs into exact f32 products, so hi+lo
            # operand pairs keep ~f32 matmul precision) ----
            hi_sb = spool.tile([BLOC, 2 * D], bf16, tag="hi")
            nc.scalar.activation(hi_sb[:], pt_sb[:], AF.Copy)
            lo_sb = spool.tile([BLOC, 2 * D], bf16, tag="lo")
            nc.vector.tensor_sub(lo_sb[:], pt_sb[:], hi_sb[:])

            # roundtrip through DRAM so per-batch rows can be re-read as
            # free-dim concats with single big DMAs
            hld = dpool.tile([BLOC, 2, 2 * D], bf16, tag="hld")
            nc.sync.dma_start(hld[:, 0, :], hi_sb[:])
            nc.scalar.dma_start(hld[:, 1, :], lo_sb[:])
            kp_dram = dpool.tile([BLOC, D], f32, tag="kp_dram")
            nc.gpsimd.dma_start(kp_dram[:], kp_sb[:])

            # ---- staging tiles for the gate matmul operands (bf16) ----
            # gate (K=6): lhsT rows [qh,ql,qh,ql,1,1] x rhs [wh,wh,wl,wl,th,tl]
            #   = (qh+ql)*(wh+wl) + th + tl ~= q*w1s + t
            lhs_sb = spool.tile([6, BLOC * D], bf16, tag="lhs")
            grhs_sb = spool.tile([6, BLOC * D], bf16, tag="grhs")
            qh_d = hld[:, 0, 0:D]
            ql_d = hld[:, 1, 0:D]
            th_d = hld[:, 0, D : 2 * D]
            tl_d = hld[:, 1, D : 2 * D]
            wbc = lambda dr: dr[0:1, :].partition_broadcast(BLOC)
            nc.sync.dma_start(lhs_sb[0:1, :], qh_d)
            nc.sync.dma_start(lhs_sb[1:2, :], ql_d)
            nc.sync.dma_start(lhs_sb[2:3, :], qh_d)
            nc.sync.dma_start(lhs_sb[3:4, :], ql_d)
            nc.gpsimd.dma_start(lhs_sb[4:5, :], ones8k[:])
            nc.gpsimd.dma_start(lhs_sb[5:6, :], ones8k[:])
            nc.scalar.dma_start(grhs_sb[0:1, :], wbc(w1sh))
            nc.scalar.dma_start(grhs_sb[1:2, :], wbc(w1sh))
            nc.scalar.dma_start(grhs_sb[2:3, :], wbc(w1sl))
            nc.scalar.dma_start(grhs_sb[3:4, :], wbc(w1sl))
            nc.scalar.dma_start(grhs_sb[4:5, :], th_d)
            nc.scalar.dma_start(grhs_sb[5:6, :], tl_d)

            # ---- qaT: per-partition exp scales.  qaT[p, r*BLOC+b] =
            # A3 * q_proj[b, 128r+p], built via PE transposes. ----
            from concourse.masks import make_identity

            ident = spool.tile([128, 128], f32, tag="ident")
            make_identity(nc, ident[:])
            qaT = spool.tile([128, NK * BLOC], f32, tag="qaT")
            with tc.tile_pool(name="tpool", bufs=2, space="PSUM") as tpool:
                for r in range(NK):
                    pst = tpool.tile([128, BLOC], f32, tag="pst")
                    nc.tensor.transpose(
                        pst[:], pt_sb[:, 128 * r : 128 * r + 128],
                        ident[0:BLOC, 0:BLOC],
                    )
                    nc.vector.tensor_scalar_mul(
                        qaT[:, r * BLOC : (r + 1) * BLOC], pst[:], A3
                    )

            # ---- main loop: quad row-chunk groups (tanh FD=4096) ----
            with (
                tc.tile_pool(name="psg", bufs=1, space="PSUM") as psg,
                tc.tile_pool(name="kbpool", bufs=2) as kbpool,
                tc.tile_pool(name="upool", bufs=2) as upool,
                tc.tile_pool(name="mpool", bufs=3) as mpool,
                tc.tile_pool(name="zpool", bufs=4) as zpool,
            ):
                for b in range(BLOC):
                    # broadcast k_proj[b, :] across all 128 partitions, twice
                    # along the free dim (pair-width custom-op operand)
                    kb = kbpool.tile([128, 2, D], f32, tag="kb")
                    kbsrc = kp_dram[b : b + 1, :].partition_broadcast(128)
                    nc.gpsimd.dma_start(kb[:, 0:1, :], kbsrc)
                    nc.gpsimd.dma_start(kb[:, 1:2, :], kbsrc)
                    kbf = kb[:].rearrange("p a f -> p (a f)")
                    for g in range(NK // 4):
                        ps_g = psg.tile([128, 4 * D], f32, tag="g")
                        for c in range(4):
                            r = 4 * g + c
                            rsl = slice(b * D + 128 * r, b * D + 128 * r + 128)
                            for nb in range(2):
                                csl = slice(
                                    b * D + 512 * nb, b * D + 512 * nb + 512
                                )
                                osl = slice(
                                    1024 * c + 512 * nb, 1024 * c + 512 * nb + 512
                                )
                                nc.tensor.matmul(
                                    ps_g[:, osl], lhs_sb[0:6, rsl],
                                    grhs_sb[0:6, csl], start=True, stop=True,
                                )
                        u = upool.tile([128, 4 * D], f32, tag="u")
                        nc.scalar.activation(u[:], ps_g[:], AF.Tanh, scale=0.5)
                        for h in range(2):
                            y = upool.tile([128, 2 * D], f32, tag="y")
                            nc.vector._custom_dve(
                                sigmul, out=y[:],
                                in0=u[:, 2048 * h : 2048 * h + 2048],
                                in1=kbf, s0=CC0, s1=CC1, imm2=CC2,
                            )
                            for c2 in range(2):
                                r = 4 * g + 2 * h + c2
                                e = mpool.tile([128, D], f32, tag="e")
                                z = zpool.tile([128, 1], f32, tag="z")
                                nc.scalar.activation(
                                    e[:], y[:, 1024 * c2 : 1024 * c2 + 1024],
                                    AF.Exp,
                                    scale=qaT[:, r * BLOC + b : r * BLOC + b + 1],
                                    accum_out=z[:],
                                )
                                rz = zpool.tile([128, 1], f32, tag="rz")
                                nc.vector.reciprocal(rz[:], z[:])
                                o = mpool.tile([128, D], f32, tag="o")
                                nc.vector.tensor_scalar_mul(o[:], e[:], rz[:])
                                (nc.sync if c2 == 0 else nc.gpsimd).dma_start(
                                    out_d[b, 128 * r : 128 * r + 128, :], o[:]
                                )

    nc.compile()
    return nc


def _prep_host(inputs):
    import ml_dtypes

    bf = ml_dtypes.bfloat16
    q = np.ascontiguousarray(np.asarray(inputs["q"], dtype=np.float32))
    k = np.ascontiguousarray(np.asarray(inputs["k"], dtype=np.float32))
    Wq = np.asarray(inputs["Wq"], dtype=np.float32)
    Wk = np.asarray(inputs["Wk"], dtype=np.float32)
    Wg = np.asarray(inputs["Wg"], dtype=np.float32)
    bq = np.asarray(inputs["bq"], dtype=np.float32)
    bk = np.asarray(inputs["bk"], dtype=np.float32)
    bg = np.asarray(inputs["bg"], dtype=np.float32)

    W1 = Wg[:, :D]
    W2 = Wg[:, D:]
    WqT = np.ascontiguousarray(Wq.T)
    WkT = np.ascontiguousarray(Wk.T)
    # t = k_proj @ W2.T + bg = k @ (W2 @ Wk).T + (bk @ W2.T + bg)
    WtT = np.ascontiguousarray((Wk.T @ W2.T).astype(bf))
    bt = (bk @ W2.T + bg).astype(np.float32).reshape(1, D)
    w1s = W1.sum(axis=1).astype(np.float32).reshape(1, D)
    w1sh = w1s.astype(bf)
    w1sl = (w1s - w1sh.astype(np.float32)).astype(bf)

    def arr(x, dt=np.float32):  # (BLOC, D) -> [p, kc*BLOC] tile layout
        return np.ascontiguousarray(
            x.T.reshape(D // 128, 128, BLOC).transpose(1, 0, 2).reshape(128, -1)
        ).astype(dt)

    shared = {
        "WqT": WqT, "WkT": WkT, "WtT": WtT,
        "w1sh": w1sh, "w1sl": w1sl,
        "ones8k": np.ones((1, BLOC * D), dtype=bf),
        "bq": bq.reshape(1, D).copy(),
        "bk": bk.reshape(1, D).copy(),
        "bt": bt,
    }
    in_maps = []
    for c in range(NCORES):
        sl = slice(c * BLOC, (c + 1) * BLOC)
        m = dict(shared)
        m["qT"] = arr(q[sl])
        m["kT"] = arr(k[sl])
        m["kTb"] = arr(k[sl], bf)
        in_maps.append(m)
    return in_maps


def kernel(**inputs) -> np.ndarray:
    global LAST_RESULTS
    from concourse.bass_utils import run_bass_kernel_spmd

    if "nc" not in _CACHE:
        _CACHE["nc"] = _build()
    nc = _CACHE["nc"]

    in_maps = _prep_host(inputs)
    res = run_bass_kernel_spmd(
        nc, in_maps, core_ids=list(range(NCORES)), trace=TRACE
    )
    LAST_RESULTS = res
    out = np.concatenate([res.results[c]["out"] for c in range(NCORES)], axis=0)
    return out
